# revision 12
# baseline (speedup 1.0000x reference)
"""Trainium2 Bass kernel for nn_Memory (space-time memory read attention).

Computation (B=4, T=8, C=256, KD=128, H=W=40):
  m_keys = conv3x3(mem,  key_w) ; m_vals = conv3x3(mem, value_w)
  q_key  = conv3x3(query, key_w); q_val  = conv3x3(query, value_w)
  p      = softmax_n( m_keys[n,d] . q_key[d,m] / sqrt(KD) )   n = T*H*W, m = H*W
  out    = concat( m_vals @ p , q_val )
Returns (mem_out (B,2*KD,H,W), p (B,T*H*W,H*W)).

Sharding: 8 cores = B(4) x T-half(2). Each core convolves its 4 mem frames,
computes its 6400-row slice of the score matrix; the softmax denominator and
value-weighted sum are completed with a pairwise AllReduce. The large p
output (328MB total) is written directly from each core's row shard.

Conv is 9 shifted matmuls over a zero-padded (42x42) SBUF frame accumulated
in PSUM; matmuls run as float32r (full-rate fp32, N>=256). The second bmm
and softmax-sum run in bf16 against the stored exp() tile.
"""
import sys, os

for _p in ("/opt/trn_rl_repo",):
    if _p not in sys.path and os.path.isdir(_p):
        sys.path.append(_p)

import numpy as np
import ml_dtypes

import concourse.bass as bass
import concourse.bacc as bacc
import concourse.mybir as mybir
import concourse.tile as tile
from concourse.bass_utils import run_bass_kernel_spmd

F32 = mybir.dt.float32
F32R = mybir.dt.float32r
BF16 = mybir.dt.bfloat16

B, T, C, H, W = 4, 8, 256, 40, 40
KD = 128          # key/val channels
HW = H * W        # 1600
T_LOC = T // 2    # frames per core
N_LOC = T_LOC * HW            # 6400 memory rows per core
NBLK = N_LOC // 128           # 50 n-blocks
MCH = 400                     # m-chunk width
NMC = HW // MCH               # 4 m-chunks
PW = W + 2                    # padded row width 42
PAD1 = PW * (H + 2)           # padded per-cin-block frame size 1764
INV_SQRT_KD = 1.0 / float(np.sqrt(KD))
N_CORES = 8
REPLICA_GROUPS = [[0, 1], [2, 3], [4, 5], [6, 7]]

_CACHE = {}


def _conv_frame(nc, psum_conv, pad_t, jobs):
    """conv3x3 of one padded SBUF frame. jobs: (w_t, b_t, dst_fn, dst_dt);
    dst_fn(ch) -> destination AP for output chunk ch (128, MCH)."""
    for w_t, b_t, dst_fn in jobs:
        for ch in range(4):           # 4 chunks of 10 spatial rows
            cp = psum_conv.tile([128, MCH], F32, tag="convp")
            r0 = ch * 10
            k = 0
            for cb in range(2):
                for ty in range(3):
                    for tx in range(3):
                        view = pad_t[:, cb * PAD1:(cb + 1) * PAD1].rearrange(
                            "p (h w) -> p h w", w=PW)
                        rhs = view[:, r0 + ty: r0 + ty + 10, tx: tx + W]
                        ti = cb * 9 + ty * 3 + tx
                        lhsT = w_t[:, ti * 128:(ti + 1) * 128]
                        nc.tensor.matmul(
                            cp[:], lhsT, rhs,
                            start=(k == 0), stop=(k == 17))
                        k += 1
            nc.scalar.activation(dst_fn(ch), cp[:],
                                 mybir.ActivationFunctionType.Identity,
                                 bias=b_t[:, 0:1], scale=1.0)


def build_program(reps=1):
    nc = bacc.Bacc("TRN2", target_bir_lowering=False, debug=False,
                   num_devices=N_CORES)

    mem4 = nc.dram_tensor("mem4", [T_LOC, C, H, W], F32R, kind="ExternalInput").ap()
    query1 = nc.dram_tensor("query1", [C, H, W], F32R, kind="ExternalInput").ap()
    kwT = nc.dram_tensor("kwT", [128, 2 * 9 * 128], F32R, kind="ExternalInput").ap()
    vwT = nc.dram_tensor("vwT", [128, 2 * 9 * 128], F32R, kind="ExternalInput").ap()
    kb = nc.dram_tensor("kb", [128, 1], F32, kind="ExternalInput").ap()
    vb = nc.dram_tensor("vb", [128, 1], F32, kind="ExternalInput").ap()
    ident = nc.dram_tensor("ident", [128, 128], BF16, kind="ExternalInput").ap()
    padzero = nc.dram_tensor("padzero", [128, 2 * PAD1], F32R,
                             kind="ExternalInput").ap()

    p_out = nc.dram_tensor("p_out", [N_LOC, HW], F32, kind="ExternalOutput").ap()
    memout = nc.dram_tensor("memout", [2 * KD, HW], F32, kind="ExternalOutput").ap()

    with tile.TileContext(nc) as tc:
        with (
            tc.tile_pool(name="persist", bufs=1) as pers,
            tc.tile_pool(name="work", bufs=3) as work,
            tc.tile_pool(name="dram", bufs=2, space="DRAM") as dram,
        ):
            ones_t = pers.tile([128, 128], BF16, tag="ones")
            nc.vector.memset(ones_t[:], 1.0)

            # ---- persistent activations
            keys_t = pers.tile([128, N_LOC], F32R, tag="keys")      # (d, n)
            valsT_t = pers.tile([128, NBLK * 128], BF16, tag="valsT")  # (n_loc, d)
            qk_t = pers.tile([128, HW], F32R, tag="qk")
            qv_t = pers.tile([128, HW], F32, tag="qv")
            zrec_t = pers.tile([128, HW], F32, tag="zrec")

            with (
                tc.tile_pool(name="convpool", bufs=1) as convp,
                tc.tile_pool(name="psum_conv", bufs=2, space="PSUM") as psum_conv,
            ):
                # ---- conv-phase-only tiles
                wk_t = convp.tile([128, 2 * 9 * 128], F32R, tag="wk")
                wv_t = convp.tile([128, 2 * 9 * 128], F32R, tag="wv")
                kb_t = convp.tile([128, 1], F32, tag="kb")
                vb_t = convp.tile([128, 1], F32, tag="vb")
                id_t = convp.tile([128, 128], BF16, tag="id")
                valsb_t = convp.tile([128, N_LOC], BF16, tag="valsb")  # (d, n)
                nc.sync.dma_start(wk_t[:], kwT)
                nc.sync.dma_start(wv_t[:], vwT)
                nc.sync.dma_start(kb_t[:], kb)
                nc.sync.dma_start(vb_t[:], vb)
                nc.sync.dma_start(id_t[:], ident)

                pad_ts = [convp.tile([128, 2 * PAD1], F32R, tag=f"pad{i}",
                                     name=f"pad{i}")
                          for i in range(2)]
                for pt in pad_ts:
                    nc.sync.dma_start(pt[:], padzero)

                def load_frame(src3d, pt):
                    for cb in range(2):
                        dst = pt[:, cb * PAD1 + PW + 1:
                                 cb * PAD1 + PW + 1 + H * PW]
                        dst = dst.rearrange("p (h w) -> p h w", w=PW)[:, :, :W]
                        nc.sync.dma_start(dst, src3d[cb * 128:(cb + 1) * 128])

                # ---- query conv -> q_key, q_val
                load_frame(query1, pad_ts[0])
                _conv_frame(nc, psum_conv, pad_ts[0], [
                    (wk_t, kb_t, lambda ch: qk_t[:, ch * MCH:(ch + 1) * MCH]),
                    (wv_t, vb_t, lambda ch: qv_t[:, ch * MCH:(ch + 1) * MCH]),
                ])
                nc.sync.dma_start(memout[KD:2 * KD, :], qv_t[:])

                # ---- memory frame convs -> keys (f32), vals (bf16)
                for f in range(T_LOC):
                    pt = pad_ts[(f + 1) % 2]
                    load_frame(mem4[f], pt)
                    off = f * HW
                    _conv_frame(nc, psum_conv, pt, [
                        (wk_t, kb_t, lambda ch, off=off:
                            keys_t[:, off + ch * MCH: off + (ch + 1) * MCH]),
                        (wv_t, vb_t, lambda ch, off=off:
                            valsb_t[:, off + ch * MCH: off + (ch + 1) * MCH]),
                    ])

                # ---- transpose vals (d,n) -> valsT (n_loc, d)
                for i in range(NBLK):
                    tp = psum_conv.tile([128, 128], BF16, tag="tpsum")
                    nc.tensor.transpose(
                        tp[:], valsb_t[:, i * 128:(i + 1) * 128], id_t[:])
                    nc.scalar.activation(valsT_t[:, i * 128:(i + 1) * 128],
                                         tp[:],
                                         mybir.ActivationFunctionType.Copy)

            # ---- attention, m-chunked
            with (
                tc.tile_pool(name="expp", bufs=2) as expp,
                tc.tile_pool(name="psum_a", bufs=2, space="PSUM") as psum_a,
            ):
                for mc in range(NMC):
                    ms = mc * MCH
                    exp_t = expp.tile([128, NBLK * MCH], BF16, tag="exps")
                    u_ps = psum_a.tile([128, MCH], F32, tag="upsum", bufs=1)
                    z_ps = psum_a.tile([128, MCH], F32, tag="zpsum", bufs=1)
                    for i in range(NBLK):
                        sp = psum_a.tile([128, MCH], F32, tag="spsum")
                        nc.tensor.matmul(
                            sp[:],
                            keys_t[:, i * 128:(i + 1) * 128],
                            qk_t[:, ms:ms + MCH],
                            start=True, stop=True)
                        es = exp_t[:, i * MCH:(i + 1) * MCH]
                        nc.scalar.activation(es, sp[:],
                                             mybir.ActivationFunctionType.Exp,
                                             scale=INV_SQRT_KD)
                        nc.tensor.matmul(u_ps[:],
                                         valsT_t[:, i * 128:(i + 1) * 128],
                                         es, start=(i == 0),
                                         stop=(i == NBLK - 1))
                        nc.tensor.matmul(z_ps[:], ones_t[:], es,
                                         start=(i == 0), stop=(i == NBLK - 1))

                    # pairwise allreduce of Z row + U chunk
                    zrow = work.tile([1, MCH], F32, tag="zrow")
                    usb = work.tile([128, MCH], F32, tag="usb")
                    nc.vector.tensor_copy(zrow[:], z_ps[0:1, :])
                    nc.scalar.activation(usb[:], u_ps[:],
                                         mybir.ActivationFunctionType.Copy)
                    arin = dram.tile([129, MCH], F32, tag="arin")
                    arout = dram.tile([129, MCH], F32, tag="arout")
                    nc.sync.dma_start(arin[0:128, :], usb[:])
                    nc.sync.dma_start(arin[128:129, :], zrow[:])
                    nc.gpsimd.collective_compute(
                        "AllReduce", mybir.AluOpType.add,
                        replica_groups=REPLICA_GROUPS,
                        ins=[arin.opt()],
                        outs=[arout.opt()])
                    zt = work.tile([1, MCH], F32, tag="zt")
                    ut = work.tile([128, MCH], F32, tag="ut")
                    nc.sync.dma_start(zt[:], arout[128:129, :])
                    nc.sync.dma_start(ut[:], arout[0:128, :])

                    # zrec chunk = broadcast(1/Z)
                    zrr = work.tile([1, MCH], F32, tag="zrr")
                    nc.vector.reciprocal(zrr[:], zt[:])
                    nc.gpsimd.partition_broadcast(zrec_t[:, ms:ms + MCH],
                                                  zrr[:])

                    # mem_read chunk = U * (1/Z)
                    mr = work.tile([128, MCH], F32, tag="mr")
                    nc.vector.tensor_mul(mr[:], ut[:], zrec_t[:, ms:ms + MCH])
                    nc.sync.dma_start(memout[0:KD, ms:ms + MCH], mr[:])

                    # normalize + write p row-blocks
                    for i in range(NBLK):
                        pn = work.tile([128, MCH], F32, tag="pnorm")
                        nc.vector.tensor_mul(pn[:],
                                             exp_t[:, i * MCH:(i + 1) * MCH],
                                             zrec_t[:, ms:ms + MCH])
                        nc.sync.dma_start(
                            p_out[i * 128:(i + 1) * 128, ms:ms + MCH], pn[:])

    nc.compile()
    return nc


def _prep_weight(w):
    # w (KD=128, C=256, 3, 3) -> (128 ci, [cb(2) x tap(9) x co(128)])
    a = w.transpose(1, 2, 3, 0)              # (256, 3, 3, 128)
    a = a.reshape(2, 128, 9, 128)            # (cb, ci, tap, co)
    a = a.transpose(1, 0, 2, 3)              # (ci, cb, tap, co)
    return np.ascontiguousarray(a.reshape(128, 2 * 9 * 128), dtype=np.float32)


def kernel(mem, query, key_w, key_b, value_w, value_b):
    mem = np.asarray(mem, np.float32)
    query = np.asarray(query, np.float32)
    if "nc" not in _CACHE:
        _CACHE["nc"] = build_program()
    nc = _CACHE["nc"]

    kwT = _prep_weight(np.asarray(key_w, np.float32))
    vwT = _prep_weight(np.asarray(value_w, np.float32))
    kb = np.asarray(key_b, np.float32).reshape(128, 1)
    vb = np.asarray(value_b, np.float32).reshape(128, 1)
    ident = np.eye(128, dtype=ml_dtypes.bfloat16)
    padzero_np = np.zeros((128, 2 * PAD1), np.float32)

    in_maps = []
    for core in range(N_CORES):
        b, half = core // 2, core % 2
        in_maps.append({
            "mem4": np.ascontiguousarray(mem[b, half * T_LOC:(half + 1) * T_LOC]),
            "query1": np.ascontiguousarray(query[b]),
            "kwT": kwT, "vwT": vwT, "kb": kb, "vb": vb, "ident": ident,
            "padzero": padzero_np,
        })

    res = run_bass_kernel_spmd(nc, in_maps, core_ids=list(range(N_CORES)))

    p = np.empty((B, T * HW, HW), np.float32)
    mem_out = np.empty((B, 2 * KD, H, W), np.float32)
    for b in range(B):
        p[b, :N_LOC] = res.results[2 * b]["p_out"]
        p[b, N_LOC:] = res.results[2 * b + 1]["p_out"]
        mem_out[b] = res.results[2 * b]["memout"].reshape(2 * KD, H, W)
    return (mem_out, p)
